# revision 5
# baseline (speedup 1.0000x reference)
"""Causal self-attention (B=4, T=2048, C=768, H=6, D=128) on 8 trn2 NeuronCores.

Sharding: 24 (batch, head) units -> 8 cores, each core owns 1 batch x 3 heads.
Per core: QKV projections for its 3 heads, RoPE + per-head norm, causal
attention, partial output projection over its heads' columns.
Unshard: out[b] = partial[core 2b] + partial[core 2b+1]  (tensor-parallel sum).

v3 (bf16 + layout/scheduling rework; the v2 trace showed ACT-table thrashing
between Ln and Exp, a DVE-bound stage 1 starving the PE, and 13us of strided
input-DMA startup):
  - every matmul operand is bf16 (1 cyc/col streaming, FWL weight loads);
    PSUM accumulation stays fp32. rel-err vs the fp32 reference ~6e-3,
    gate is 2e-2.
  - all inputs are host-swizzled into partition-major SBUF images so every
    DMA line is contiguous-per-partition (1.5-4.6KB lines, no 256B scatter).
  - rope via host tables CC=[cos|cos], SS=[sin|-sin]: 4 bf16 DVE ops per
    (tile, q/k) instead of 6 fp32 ones.
  - stage 2 stats are BATCHED: loop A does QKV+rope+bn_stats for all 16
    token tiles, ONE combine computes rstd/nmrs for all tiles (so Ln and
    Exp each load their ACT table once instead of 16 interleaved reloads),
    loop B does norm-apply + transposes.  rstd = exp(-0.5*ln(var/127));
    eps=1e-6 is dropped (std ~ 0.55, relative effect 2e-6).
  - norm-apply = one tensor_scalar (r*rstd + (-mean*rstd)) per head.
  - Q/K transposes go through the DMA xbar (dma_start_transpose) instead of
    the PE, freeing ~21us of tensor-engine time and dropping the PSUM->SBUF
    copy after each PE transpose.
  - exp over PAIRED score blocks [128, 1024] (fewer 352-cycle ACT overheads).
  - causal mask via a host [128, 896] 0/1 bf16 sliding-window table: one DVE
    multiply per diagonal block.
  - softmax denominator reciprocal via reciprocal_approx_fast.
  - partial outputs leave the device in bf16; host sums core pairs in fp32.
"""

import ml_dtypes
import numpy as np

import concourse.bacc as bacc
import concourse.bass as bass
import concourse.mybir as mybir
from concourse import tile
from concourse.bass_utils import run_bass_kernel_spmd

F32 = mybir.dt.float32
BF16 = mybir.dt.bfloat16
AF = mybir.ActivationFunctionType
ALU = mybir.AluOpType

B, T, C, H, D = 4, 2048, 768, 6, 128
HALF = D // 2
NH = 3            # heads per core
CT = C // 128     # 6 contraction tiles for projections
NT = T // 128     # 16 token tiles
QC = 512          # query-chunk width for attention
NQC = T // QC     # 4 chunks
SCALE = 1.0 / float(np.sqrt(D))
PE_TRANSPOSE = False  # fallback: PE-based transposes instead of DMA xbar

_CACHE = {}


def _build_nc():
    nc = bacc.Bacc("TRN2")

    xh = nc.dram_tensor("xh", [128, NT, CT, 128], BF16, kind="ExternalInput")
    wqh = nc.dram_tensor("wqh", [128, CT, NH * D], BF16, kind="ExternalInput")
    wkh = nc.dram_tensor("wkh", [128, CT, NH * D], BF16, kind="ExternalInput")
    wvh = nc.dram_tensor("wvh", [128, CT, NH * D], BF16, kind="ExternalInput")
    wph = nc.dram_tensor("wph", [128, NH, C], BF16, kind="ExternalInput")
    cch = nc.dram_tensor("cch", [128, NT, NH * D], BF16, kind="ExternalInput")
    ssh = nc.dram_tensor("ssh", [128, NT, NH * D], BF16, kind="ExternalInput")
    ident = nc.dram_tensor("ident", [128, 128], BF16, kind="ExternalInput")
    ones_in = nc.dram_tensor("ones_in", [128, 1], BF16, kind="ExternalInput")
    mask_in = nc.dram_tensor("mask_in", [128, 384 + QC], BF16, kind="ExternalInput")
    out = nc.dram_tensor("out", [T, C], BF16, kind="ExternalOutput")

    with tile.TileContext(nc) as tc:
        with (
            tc.tile_pool(name="persist", bufs=1) as persist,
            tc.tile_pool(name="qkvbuf", bufs=1) as qkvbuf,
        ):
            QT = qkvbuf.tile([128, NH, T], BF16)       # [d, h, t]
            KT = qkvbuf.tile([128, NH, T], BF16)       # [d, h, t]
            V = qkvbuf.tile([128, NT, NH * D], BF16)   # [s%128, s//128, h*D+d]
            RQ = qkvbuf.tile([128, NT, NH * D], BF16)  # rope(q), pre-norm
            RK = qkvbuf.tile([128, NT, NH * D], BF16)
            ones = persist.tile([128, 1], BF16)
            idn = persist.tile([128, 128], BF16)
            wp_sb = persist.tile([128, NH, C], BF16)   # [d, h, c]
            msk = persist.tile([128, 384 + QC], BF16)

            # ---------------- stage 1+2: QKV projection + rope + norm ---------
            with (
                tc.tile_pool(name="wbuf", bufs=1) as wbuf,
                tc.tile_pool(name="xch", bufs=3) as xpool,
                tc.tile_pool(name="rope", bufs=4) as rpool,
                tc.tile_pool(name="stat", bufs=1) as spool,
                tc.tile_pool(name="psA", bufs=3, space="PSUM") as psA,
                tc.tile_pool(name="psT", bufs=2, space="PSUM") as psT,
            ):
                wq_sb = wbuf.tile([128, CT, NH * D], BF16)
                wk_sb = wbuf.tile([128, CT, NH * D], BF16)
                wv_sb = wbuf.tile([128, CT, NH * D], BF16)
                # startup-latency ordering: first-tile deps first
                nc.sync.dma_start(wq_sb[:], wqh[:])
                nc.sync.dma_start(wk_sb[:], wkh[:])
                nc.sync.dma_start(wv_sb[:], wvh[:])

                xch0 = xpool.tile([128, CT, 128], BF16, tag="xch")
                nc.sync.dma_start(xch0[:], xh[:, 0])

                cc_sb = wbuf.tile([128, NT, NH * D], BF16)
                ss_sb = wbuf.tile([128, NT, NH * D], BF16)
                nc.sync.dma_start(cc_sb[:], cch[:])
                nc.sync.dma_start(ss_sb[:], ssh[:])
                nc.sync.dma_start(idn[:], ident[:])
                nc.sync.dma_start(wp_sb[:], wph[:])
                nc.sync.dma_start(ones[:], ones_in[:])
                nc.sync.dma_start(msk[:], mask_in[:])

                # stats for all tiles: [tile, q/k, head, bn6]
                Sall = spool.tile([128, NT, 2, NH, 6], F32)

                # ---- loop A: projections + rope + bn_stats -------------------
                for tt in range(NT):
                    if tt == 0:
                        xch = xch0
                    else:
                        xch = xpool.tile([128, CT, 128], BF16, tag="xch")
                        nc.sync.dma_start(xch[:], xh[:, tt])

                    qps = psA.tile([128, NH * D], F32, tag="ps")
                    kps = psA.tile([128, NH * D], F32, tag="ps")
                    vps = psA.tile([128, NH * D], F32, tag="ps")
                    for ci in range(CT):
                        st_, sp_ = (ci == 0), (ci == CT - 1)
                        lhs = xch[:, ci, :]
                        nc.tensor.matmul(qps[:], lhs, wq_sb[:, ci, :], start=st_, stop=sp_)
                        nc.tensor.matmul(kps[:], lhs, wk_sb[:, ci, :], start=st_, stop=sp_)
                        nc.tensor.matmul(vps[:], lhs, wv_sb[:, ci, :], start=st_, stop=sp_)

                    # V: straight copy PSUM -> SBUF bf16 in natural [t, o] layout
                    nc.scalar.copy(V[:, tt, :], vps[:])

                    for mi, ps, R in ((0, qps, RQ), (1, kps, RK)):
                        sb = rpool.tile([128, NH * D], BF16, tag=f"sb{mi}")
                        nc.scalar.copy(sb[:], ps[:])
                        sb_v = sb[:].rearrange("p (h d) -> p h d", h=NH)
                        r = R[:, tt, :]
                        r_v = r.rearrange("p (h d) -> p h d", h=NH)
                        t2 = rpool.tile([128, NH * D], BF16, tag=f"t2{mi}")
                        t2_v = t2[:].rearrange("p (h d) -> p h d", h=NH)
                        # rope: r = u*CC + swap(u)*SS, swap done by half-slices
                        nc.vector.tensor_mul(
                            t2_v[:, :, 0:HALF], sb_v[:, :, HALF:D], ss_sb[:, tt].rearrange("p (h d) -> p h d", h=NH)[:, :, 0:HALF])
                        nc.vector.tensor_mul(
                            t2_v[:, :, HALF:D], sb_v[:, :, 0:HALF], ss_sb[:, tt].rearrange("p (h d) -> p h d", h=NH)[:, :, HALF:D])
                        nc.vector.tensor_mul(r, sb[:], cc_sb[:, tt, :])
                        nc.vector.tensor_add(r, r, t2[:])
                        for h in range(NH):
                            nc.vector.bn_stats(Sall[:, tt, mi, h], r_v[:, h])

                # ---- one combine for all tiles: rstd/nmrs --------------------
                # var*128 = cv_e + cv_o + 32*(m_e - m_o)^2   (ddof=1 -> /127)
                G = NT * 2 * NH
                dm = spool.tile([128, G], F32)
                ms = spool.tile([128, G], F32)
                cv = spool.tile([128, G], F32)
                s2 = spool.tile([128, G], F32)
                var = spool.tile([128, G], F32)
                lnv = spool.tile([128, G], F32)
                rstd = spool.tile([128, G], F32)
                nmrs = spool.tile([128, G], F32)
                m_e = Sall[:, :, :, :, 1]
                m_o = Sall[:, :, :, :, 4]
                cv_e = Sall[:, :, :, :, 2]
                cv_o = Sall[:, :, :, :, 5]
                sh = dict(a=NT, b=2)
                dm_v = dm[:].rearrange("p (a b c) -> p a b c", **sh)
                ms_v = ms[:].rearrange("p (a b c) -> p a b c", **sh)
                cv_v = cv[:].rearrange("p (a b c) -> p a b c", **sh)
                nc.vector.tensor_sub(dm_v, m_e, m_o)
                nc.vector.tensor_add(ms_v, m_e, m_o)
                nc.vector.tensor_add(cv_v, cv_e, cv_o)
                nc.vector.scalar_tensor_tensor(
                    s2[:], dm[:], 32.0, dm[:], op0=ALU.mult, op1=ALU.mult)
                nc.vector.tensor_add(var[:], cv[:], s2[:])
                nc.scalar.activation(lnv[:], var[:], AF.Ln, scale=1.0 / (D - 1))
                nc.scalar.activation(rstd[:], lnv[:], AF.Exp, scale=-0.5)
                nc.vector.scalar_tensor_tensor(
                    nmrs[:], ms[:], -0.5, rstd[:], op0=ALU.mult, op1=ALU.mult)
                rstd_v = rstd[:].rearrange("p (a b c) -> p a b c", **sh)
                nmrs_v = nmrs[:].rearrange("p (a b c) -> p a b c", **sh)

                # ---- loop B: norm-apply + transpose to [d, t] ----------------
                for tt in range(NT):
                    for mi, R, dstT in ((0, RQ, QT), (1, RK, KT)):
                        nrm = rpool.tile([128, NH * D], BF16, tag=f"n{mi}")
                        if PE_TRANSPOSE:
                            tps = psT.tile([128, NH * D], BF16, tag="tp")
                        for h in range(NH):
                            nc.vector.tensor_scalar(
                                nrm[:, h * D:(h + 1) * D],
                                R[:, tt, h * D:(h + 1) * D],
                                rstd_v[:, tt, mi, h:h + 1],
                                nmrs_v[:, tt, mi, h:h + 1],
                                ALU.mult,
                                ALU.add,
                            )
                            if PE_TRANSPOSE:
                                nc.tensor.transpose(tps[:, h * D:(h + 1) * D],
                                                    nrm[:, h * D:(h + 1) * D], idn[:])
                            else:
                                nc.sync.dma_start_transpose(
                                    dstT[:, h, tt * 128:(tt + 1) * 128],
                                    nrm[:, h * D:(h + 1) * D])
                        if PE_TRANSPOSE:
                            dst = dstT[:, :, tt * 128:(tt + 1) * 128]
                            src = tps[:].rearrange("p (h t) -> p h t", h=NH)
                            nc.vector.tensor_copy(dst, src)

            # ---------------- stage 3+4: attention + output projection --------
            with (
                tc.tile_pool(name="att", bufs=3) as apool,
                tc.tile_pool(name="acc", bufs=2) as accpool,
                tc.tile_pool(name="ybuf", bufs=2) as ypool,
                tc.tile_pool(name="obuf", bufs=3) as opool,
                tc.tile_pool(name="psPair", bufs=2, space="PSUM") as psPair,
                tc.tile_pool(name="psY", bufs=2, space="PSUM") as psY,
                tc.tile_pool(name="psD", bufs=1, space="PSUM") as psD,
                tc.tile_pool(name="psP", bufs=1, space="PSUM") as psP,
            ):
                out_r = out.rearrange("(tt p) c -> p tt c", p=128)

                def emit_proj(qc, yTc):
                    # output projection for chunk qc's 4 token tiles
                    for j in range(QC // 128):
                        tt = qc * (QC // 128) + j
                        ot = opool.tile([128, C], BF16, tag="ot")
                        for half in range(2):
                            op_ps = psP.tile([128, C // 2], F32, tag="pp")
                            csl = slice(half * (C // 2), (half + 1) * (C // 2))
                            for h in range(NH):
                                lhs = yTc[:, h, j * 128:(j + 1) * 128]
                                nc.tensor.matmul(op_ps[:], lhs, wp_sb[:, h, csl],
                                                 start=(h == 0), stop=(h == NH - 1))
                            nc.scalar.copy(ot[:, csl], op_ps[:])
                        nc.sync.dma_start(out_r[:, tt, :], ot[:])

                pending = None
                for qc in range(NQC):
                    Q0 = qc * QC
                    n_st = (Q0 + QC) // 128
                    yTc = ypool.tile([128, NH, QC], BF16, tag="yT")  # [d, h, q]
                    for h in range(NH):
                        yps = psY.tile([128, QC], F32, tag="yps")
                        dps = psD.tile([1, QC], F32, tag="dps")
                        for sp in range(n_st // 2):
                            pair = psPair.tile([128, 2 * QC], F32, tag="pair")
                            for j in range(2):
                                st = 2 * sp + j
                                nc.tensor.matmul(
                                    pair[:, j * QC:(j + 1) * QC],
                                    KT[:, h, st * 128:(st + 1) * 128],
                                    QT[:, h, Q0:Q0 + QC],
                                    start=True, stop=True,
                                )
                            et = apool.tile([128, 2 * QC], BF16, tag="et")
                            nc.scalar.activation(et[:], pair[:], AF.Exp, scale=SCALE)
                            for j in range(2):
                                st = 2 * sp + j
                                k = st - n_st + 4
                                if k >= 0:  # diagonal block: zero where s > q
                                    esl = et[:, j * QC:(j + 1) * QC]
                                    nc.vector.tensor_mul(
                                        esl, esl, msk[:, 384 - 128 * k:896 - 128 * k])
                            for j in range(2):
                                st = 2 * sp + j
                                esl = et[:, j * QC:(j + 1) * QC]
                                nc.tensor.matmul(
                                    yps[:],
                                    V[:, st, h * D:(h + 1) * D],
                                    esl,
                                    start=(st == 0), stop=(st == n_st - 1),
                                    skip_group_check=True,
                                )
                                # softmax denominator on PE: ones^T @ exp
                                nc.tensor.matmul(
                                    dps[:1, :],
                                    ones[:],
                                    esl,
                                    start=(st == 0), stop=(st == n_st - 1),
                                    skip_group_check=True,
                                )
                        rc1 = accpool.tile([1, QC], F32, tag="rc1")
                        nc.vector.reciprocal_approx_fast(rc1[:1, :], dps[:1, :])
                        rbc = accpool.tile([128, QC], F32, tag="rbc")
                        nc.gpsimd.partition_broadcast(rbc[:], rc1[:1, :])
                        nc.vector.tensor_mul(yTc[:, h, :], yps[:], rbc[:])
                        if h == 0 and pending is not None:
                            # previous chunk's projection lands here so its
                            # yTc-normalize latency hides under this chunk's
                            # independent attention matmuls
                            emit_proj(*pending)
                            pending = None

                    pending = (qc, yTc)
                emit_proj(*pending)

    nc.compile()
    return nc


def _get_nc():
    if "nc" not in _CACHE:
        _CACHE["nc"] = _build_nc()
    return _CACHE["nc"]


def _bf16(a):
    return np.ascontiguousarray(np.asarray(a, np.float32)).astype(ml_dtypes.bfloat16)


def _in_maps(x, cos, sin, wq, wk, wv, wproj):
    cos = np.asarray(cos, np.float32)
    sin = np.asarray(sin, np.float32)
    cc = np.tile(np.concatenate([cos, cos], axis=1), (1, NH))      # [T, NH*D]
    ss = np.tile(np.concatenate([sin, -sin], axis=1), (1, NH))     # [T, NH*D]
    # partition-major images: contiguous per-partition DMA lines
    cch = np.ascontiguousarray(cc.reshape(NT, 128, NH * D).transpose(1, 0, 2))
    ssh = np.ascontiguousarray(ss.reshape(NT, 128, NH * D).transpose(1, 0, 2))
    ident = np.eye(128, dtype=np.float32)
    # msk[p, u] = 1 iff u >= p + 384; diagonal block k uses cols [384-128k, ...)
    p = np.arange(128)[:, None]
    u = np.arange(384 + QC)[None, :]
    mask = (u >= p + 384).astype(np.float32)

    x = np.asarray(x, np.float32)
    wq = np.asarray(wq, np.float32)
    wk = np.asarray(wk, np.float32)
    wv = np.asarray(wv, np.float32)
    wpT = np.asarray(wproj, np.float32).T

    maps = []
    for c in range(8):
        b = c // 2
        hs = (c % 2) * NH
        sl = slice(hs * D, (hs + NH) * D)
        # xh[p, tt, ci, u] = x[b][tt*128+u, ci*128+p]
        xhost = x[b].reshape(NT, 128, CT, 128).transpose(3, 0, 2, 1)
        # wqh[p, ci, o] = wq[sl][o, ci*128+p]
        wqhost = wq[sl].T.reshape(CT, 128, NH * D).transpose(1, 0, 2)
        wkhost = wk[sl].T.reshape(CT, 128, NH * D).transpose(1, 0, 2)
        wvhost = wv[sl].T.reshape(CT, 128, NH * D).transpose(1, 0, 2)
        # wph[p, h, c] = wproj.T[sl][h*128+p, c]
        wphost = wpT[sl].reshape(NH, 128, C).transpose(1, 0, 2)
        maps.append({
            "xh": _bf16(xhost),
            "wqh": _bf16(wqhost),
            "wkh": _bf16(wkhost),
            "wvh": _bf16(wvhost),
            "wph": _bf16(wphost),
            "cch": _bf16(cch),
            "ssh": _bf16(ssh),
            "ident": _bf16(ident),
            "ones_in": _bf16(np.ones((128, 1), dtype=np.float32)),
            "mask_in": _bf16(mask),
        })
    return maps


def kernel(x, cos, sin, wq, wk, wv, wproj, _trace=False):
    nc = _get_nc()
    maps = _in_maps(x, cos, sin, wq, wk, wv, wproj)
    res = run_bass_kernel_spmd(nc, maps, core_ids=list(range(8)), trace=_trace)
    parts = [np.asarray(r["out"], dtype=np.float32) for r in res.results]
    outv = np.stack([parts[2 * b] + parts[2 * b + 1] for b in range(B)])
    if _trace:
        _CACHE["last_results"] = res
    return outv.astype(np.float32)


# revision 6
# speedup vs baseline: 1.0142x; 1.0142x over previous
"""Causal self-attention (B=4, T=2048, C=768, H=6, D=128) on 8 trn2 NeuronCores.

Sharding: 24 (batch, head) units -> 8 cores, each core owns 1 batch x 3 heads.
Per core: QKV projections for its 3 heads, RoPE + per-head norm, causal
attention, partial output projection over its heads' columns.
Unshard: out[b] = partial[core 2b] + partial[core 2b+1]  (tensor-parallel sum).

v3 (bf16 + layout/scheduling rework; the v2 trace showed ACT-table thrashing
between Ln and Exp, a DVE-bound stage 1 starving the PE, and 13us of strided
input-DMA startup):
  - every matmul operand is bf16 (1 cyc/col streaming, FWL weight loads);
    PSUM accumulation stays fp32. rel-err vs the fp32 reference ~6e-3,
    gate is 2e-2.
  - all inputs are host-swizzled into partition-major SBUF images so every
    DMA line is contiguous-per-partition (1.5-4.6KB lines, no 256B scatter).
  - rope via host tables CC=[cos|cos], SS=[sin|-sin]: 4 bf16 DVE ops per
    (tile, q/k) instead of 6 fp32 ones.
  - stage 2 stats are BATCHED: loop A does QKV+rope+bn_stats for all 16
    token tiles, ONE combine computes rstd/nmrs for all tiles (so Ln and
    Exp each load their ACT table once instead of 16 interleaved reloads),
    loop B does norm-apply + transposes.  rstd = exp(-0.5*ln(var/127));
    eps=1e-6 is dropped (std ~ 0.55, relative effect 2e-6).
  - norm-apply = one tensor_scalar (r*rstd + (-mean*rstd)) per head.
  - Q/K transposes go through the DMA xbar (dma_start_transpose) instead of
    the PE, freeing ~21us of tensor-engine time and dropping the PSUM->SBUF
    copy after each PE transpose.
  - exp over PAIRED score blocks [128, 1024] (fewer 352-cycle ACT overheads).
  - causal mask via a host [128, 896] 0/1 bf16 sliding-window table: one DVE
    multiply per diagonal block.
  - softmax denominator reciprocal via reciprocal_approx_fast.
  - partial outputs leave the device in bf16; host sums core pairs in fp32.
"""

import ml_dtypes
import numpy as np

import concourse.bacc as bacc
import concourse.bass as bass
import concourse.mybir as mybir
from concourse import tile
from concourse.bass_utils import run_bass_kernel_spmd

F32 = mybir.dt.float32
BF16 = mybir.dt.bfloat16
AF = mybir.ActivationFunctionType
ALU = mybir.AluOpType

B, T, C, H, D = 4, 2048, 768, 6, 128
HALF = D // 2
NH = 3            # heads per core
CT = C // 128     # 6 contraction tiles for projections
NT = T // 128     # 16 token tiles
QC = 512          # query-chunk width for attention
NQC = T // QC     # 4 chunks
SCALE = 1.0 / float(np.sqrt(D))
PE_TRANSPOSE = True  # DMA xbar transpose measured 1191ns/op (114us serial on Sync) - PE wins

_CACHE = {}


def _build_nc():
    nc = bacc.Bacc("TRN2")

    xh = nc.dram_tensor("xh", [128, NT, CT, 128], BF16, kind="ExternalInput")
    wqh = nc.dram_tensor("wqh", [128, CT, NH * D], BF16, kind="ExternalInput")
    wkh = nc.dram_tensor("wkh", [128, CT, NH * D], BF16, kind="ExternalInput")
    wvh = nc.dram_tensor("wvh", [128, CT, NH * D], BF16, kind="ExternalInput")
    wph = nc.dram_tensor("wph", [128, NH, C], BF16, kind="ExternalInput")
    cch = nc.dram_tensor("cch", [128, NT, NH * D], BF16, kind="ExternalInput")
    ssh = nc.dram_tensor("ssh", [128, NT, NH * D], BF16, kind="ExternalInput")
    ident = nc.dram_tensor("ident", [128, 128], BF16, kind="ExternalInput")
    ones_in = nc.dram_tensor("ones_in", [128, 1], BF16, kind="ExternalInput")
    mask_in = nc.dram_tensor("mask_in", [128, 384 + QC], BF16, kind="ExternalInput")
    out = nc.dram_tensor("out", [T, C], BF16, kind="ExternalOutput")

    with tile.TileContext(nc) as tc:
        with (
            tc.tile_pool(name="persist", bufs=1) as persist,
            tc.tile_pool(name="qkvbuf", bufs=1) as qkvbuf,
        ):
            QT = qkvbuf.tile([128, NH, T], BF16)       # [d, h, t]
            KT = qkvbuf.tile([128, NH, T], BF16)       # [d, h, t]
            V = qkvbuf.tile([128, NT, NH * D], BF16)   # [s%128, s//128, h*D+d]
            RQ = qkvbuf.tile([128, NT, NH * D], BF16)  # rope(q), pre-norm
            RK = qkvbuf.tile([128, NT, NH * D], BF16)
            ones = persist.tile([128, 1], BF16)
            idn = persist.tile([128, 128], BF16)
            wp_sb = persist.tile([128, NH, C], BF16)   # [d, h, c]
            msk = persist.tile([128, 384 + QC], BF16)

            # ---------------- stage 1+2: QKV projection + rope + norm ---------
            with (
                tc.tile_pool(name="wbuf", bufs=1) as wbuf,
                tc.tile_pool(name="xch", bufs=3) as xpool,
                tc.tile_pool(name="rope", bufs=4) as rpool,
                tc.tile_pool(name="stat", bufs=1) as spool,
                tc.tile_pool(name="psA", bufs=3, space="PSUM") as psA,
                tc.tile_pool(name="psT", bufs=2, space="PSUM") as psT,
            ):
                wq_sb = wbuf.tile([128, CT, NH * D], BF16)
                wk_sb = wbuf.tile([128, CT, NH * D], BF16)
                wv_sb = wbuf.tile([128, CT, NH * D], BF16)
                # startup-latency ordering: first-tile deps first
                nc.sync.dma_start(wq_sb[:], wqh[:])
                nc.sync.dma_start(wk_sb[:], wkh[:])
                nc.sync.dma_start(wv_sb[:], wvh[:])

                xch0 = xpool.tile([128, CT, 128], BF16, tag="xch")
                nc.sync.dma_start(xch0[:], xh[:, 0])

                cc_sb = wbuf.tile([128, NT, NH * D], BF16)
                ss_sb = wbuf.tile([128, NT, NH * D], BF16)
                nc.sync.dma_start(cc_sb[:], cch[:])
                nc.sync.dma_start(ss_sb[:], ssh[:])
                nc.sync.dma_start(idn[:], ident[:])
                nc.sync.dma_start(wp_sb[:], wph[:])
                nc.sync.dma_start(ones[:], ones_in[:])
                nc.sync.dma_start(msk[:], mask_in[:])

                # stats for all tiles: [tile, q/k, head, bn6]
                Sall = spool.tile([128, NT, 2, NH, 6], F32)

                # ---- loop A: projections + rope + bn_stats -------------------
                for tt in range(NT):
                    if tt == 0:
                        xch = xch0
                    else:
                        xch = xpool.tile([128, CT, 128], BF16, tag="xch")
                        nc.sync.dma_start(xch[:], xh[:, tt])

                    qps = psA.tile([128, NH * D], F32, tag="ps")
                    kps = psA.tile([128, NH * D], F32, tag="ps")
                    vps = psA.tile([128, NH * D], F32, tag="ps")
                    for ci in range(CT):
                        st_, sp_ = (ci == 0), (ci == CT - 1)
                        lhs = xch[:, ci, :]
                        nc.tensor.matmul(qps[:], lhs, wq_sb[:, ci, :], start=st_, stop=sp_)
                        nc.tensor.matmul(kps[:], lhs, wk_sb[:, ci, :], start=st_, stop=sp_)
                        nc.tensor.matmul(vps[:], lhs, wv_sb[:, ci, :], start=st_, stop=sp_)

                    # V: straight copy PSUM -> SBUF bf16 in natural [t, o] layout
                    nc.scalar.copy(V[:, tt, :], vps[:])

                    for mi, ps, R in ((0, qps, RQ), (1, kps, RK)):
                        sb = rpool.tile([128, NH * D], BF16, tag=f"sb{mi}")
                        nc.scalar.copy(sb[:], ps[:])
                        sb_v = sb[:].rearrange("p (h d) -> p h d", h=NH)
                        r = R[:, tt, :]
                        r_v = r.rearrange("p (h d) -> p h d", h=NH)
                        t2 = rpool.tile([128, NH * D], BF16, tag=f"t2{mi}")
                        t2_v = t2[:].rearrange("p (h d) -> p h d", h=NH)
                        # rope: r = u*CC + swap(u)*SS, swap done by half-slices
                        nc.vector.tensor_mul(
                            t2_v[:, :, 0:HALF], sb_v[:, :, HALF:D], ss_sb[:, tt].rearrange("p (h d) -> p h d", h=NH)[:, :, 0:HALF])
                        nc.vector.tensor_mul(
                            t2_v[:, :, HALF:D], sb_v[:, :, 0:HALF], ss_sb[:, tt].rearrange("p (h d) -> p h d", h=NH)[:, :, HALF:D])
                        nc.vector.tensor_mul(r, sb[:], cc_sb[:, tt, :])
                        nc.vector.tensor_add(r, r, t2[:])
                        for h in range(NH):
                            nc.vector.bn_stats(Sall[:, tt, mi, h], r_v[:, h])

                # ---- one combine for all tiles: rstd/nmrs --------------------
                # var*128 = cv_e + cv_o + 32*(m_e - m_o)^2   (ddof=1 -> /127)
                G = NT * 2 * NH
                dm = spool.tile([128, G], F32)
                ms = spool.tile([128, G], F32)
                cv = spool.tile([128, G], F32)
                s2 = spool.tile([128, G], F32)
                var = spool.tile([128, G], F32)
                lnv = spool.tile([128, G], F32)
                rstd = spool.tile([128, G], F32)
                nmrs = spool.tile([128, G], F32)
                m_e = Sall[:, :, :, :, 1]
                m_o = Sall[:, :, :, :, 4]
                cv_e = Sall[:, :, :, :, 2]
                cv_o = Sall[:, :, :, :, 5]
                sh = dict(a=NT, b=2)
                dm_v = dm[:].rearrange("p (a b c) -> p a b c", **sh)
                ms_v = ms[:].rearrange("p (a b c) -> p a b c", **sh)
                cv_v = cv[:].rearrange("p (a b c) -> p a b c", **sh)
                nc.vector.tensor_sub(dm_v, m_e, m_o)
                nc.vector.tensor_add(ms_v, m_e, m_o)
                nc.vector.tensor_add(cv_v, cv_e, cv_o)
                nc.vector.scalar_tensor_tensor(
                    s2[:], dm[:], 32.0, dm[:], op0=ALU.mult, op1=ALU.mult)
                nc.vector.tensor_add(var[:], cv[:], s2[:])
                nc.scalar.activation(lnv[:], var[:], AF.Ln, scale=1.0 / (D - 1))
                nc.scalar.activation(rstd[:], lnv[:], AF.Exp, scale=-0.5)
                nc.vector.scalar_tensor_tensor(
                    nmrs[:], ms[:], -0.5, rstd[:], op0=ALU.mult, op1=ALU.mult)
                rstd_v = rstd[:].rearrange("p (a b c) -> p a b c", **sh)
                nmrs_v = nmrs[:].rearrange("p (a b c) -> p a b c", **sh)

                # ---- loop B: norm-apply + transpose to [d, t] ----------------
                for tt in range(NT):
                    for mi, R, dstT in ((0, RQ, QT), (1, RK, KT)):
                        nrm = rpool.tile([128, NH * D], BF16, tag=f"n{mi}")
                        if PE_TRANSPOSE:
                            tps = psT.tile([128, NH * D], BF16, tag="tp")
                        for h in range(NH):
                            nc.vector.tensor_scalar(
                                nrm[:, h * D:(h + 1) * D],
                                R[:, tt, h * D:(h + 1) * D],
                                rstd_v[:, tt, mi, h:h + 1],
                                nmrs_v[:, tt, mi, h:h + 1],
                                ALU.mult,
                                ALU.add,
                            )
                            if PE_TRANSPOSE:
                                nc.tensor.transpose(tps[:, h * D:(h + 1) * D],
                                                    nrm[:, h * D:(h + 1) * D], idn[:])
                            else:
                                nc.sync.dma_start_transpose(
                                    dstT[:, h, tt * 128:(tt + 1) * 128],
                                    nrm[:, h * D:(h + 1) * D])
                        if PE_TRANSPOSE:
                            dst = dstT[:, :, tt * 128:(tt + 1) * 128]
                            src = tps[:].rearrange("p (h t) -> p h t", h=NH)
                            nc.vector.tensor_copy(dst, src)

            # ---------------- stage 3+4: attention + output projection --------
            with (
                tc.tile_pool(name="att", bufs=3) as apool,
                tc.tile_pool(name="acc", bufs=2) as accpool,
                tc.tile_pool(name="ybuf", bufs=2) as ypool,
                tc.tile_pool(name="obuf", bufs=3) as opool,
                tc.tile_pool(name="psPair", bufs=2, space="PSUM") as psPair,
                tc.tile_pool(name="psY", bufs=2, space="PSUM") as psY,
                tc.tile_pool(name="psD", bufs=1, space="PSUM") as psD,
                tc.tile_pool(name="psP", bufs=1, space="PSUM") as psP,
            ):
                out_r = out.rearrange("(tt p) c -> p tt c", p=128)

                def emit_proj(qc, yTc):
                    # output projection for chunk qc's 4 token tiles
                    for j in range(QC // 128):
                        tt = qc * (QC // 128) + j
                        ot = opool.tile([128, C], BF16, tag="ot")
                        for half in range(2):
                            op_ps = psP.tile([128, C // 2], F32, tag="pp")
                            csl = slice(half * (C // 2), (half + 1) * (C // 2))
                            for h in range(NH):
                                lhs = yTc[:, h, j * 128:(j + 1) * 128]
                                nc.tensor.matmul(op_ps[:], lhs, wp_sb[:, h, csl],
                                                 start=(h == 0), stop=(h == NH - 1))
                            nc.vector.tensor_copy(ot[:, csl], op_ps[:])
                        nc.sync.dma_start(out_r[:, tt, :], ot[:])

                pending = None
                for qc in range(NQC):
                    Q0 = qc * QC
                    n_st = (Q0 + QC) // 128
                    yTc = ypool.tile([128, NH, QC], BF16, tag="yT")  # [d, h, q]
                    for h in range(NH):
                        yps = psY.tile([128, QC], F32, tag="yps")
                        dps = psD.tile([1, QC], F32, tag="dps")
                        for sp in range(n_st // 2):
                            pair = psPair.tile([128, 2 * QC], F32, tag="pair")
                            for j in range(2):
                                st = 2 * sp + j
                                nc.tensor.matmul(
                                    pair[:, j * QC:(j + 1) * QC],
                                    KT[:, h, st * 128:(st + 1) * 128],
                                    QT[:, h, Q0:Q0 + QC],
                                    start=True, stop=True,
                                )
                            et = apool.tile([128, 2 * QC], BF16, tag="et")
                            nc.scalar.activation(et[:], pair[:], AF.Exp, scale=SCALE)
                            for j in range(2):
                                st = 2 * sp + j
                                k = st - n_st + 4
                                if k >= 0:  # diagonal block: zero where s > q
                                    esl = et[:, j * QC:(j + 1) * QC]
                                    nc.vector.tensor_mul(
                                        esl, esl, msk[:, 384 - 128 * k:896 - 128 * k])
                            for j in range(2):
                                st = 2 * sp + j
                                esl = et[:, j * QC:(j + 1) * QC]
                                nc.tensor.matmul(
                                    yps[:],
                                    V[:, st, h * D:(h + 1) * D],
                                    esl,
                                    start=(st == 0), stop=(st == n_st - 1),
                                    skip_group_check=True,
                                )
                                # softmax denominator on PE: ones^T @ exp
                                nc.tensor.matmul(
                                    dps[:1, :],
                                    ones[:],
                                    esl,
                                    start=(st == 0), stop=(st == n_st - 1),
                                    skip_group_check=True,
                                )
                        rc1 = accpool.tile([1, QC], F32, tag="rc1")
                        nc.vector.reciprocal_approx_fast(rc1[:1, :], dps[:1, :])
                        rbc = accpool.tile([128, QC], F32, tag="rbc")
                        nc.gpsimd.partition_broadcast(rbc[:], rc1[:1, :])
                        nc.vector.tensor_mul(yTc[:, h, :], yps[:], rbc[:])
                        if h == 0 and pending is not None:
                            # previous chunk's projection lands here so its
                            # yTc-normalize latency hides under this chunk's
                            # independent attention matmuls
                            emit_proj(*pending)
                            pending = None

                    pending = (qc, yTc)
                emit_proj(*pending)

    nc.compile()
    return nc


def _get_nc():
    if "nc" not in _CACHE:
        _CACHE["nc"] = _build_nc()
    return _CACHE["nc"]


def _bf16(a):
    return np.ascontiguousarray(np.asarray(a, np.float32)).astype(ml_dtypes.bfloat16)


def _in_maps(x, cos, sin, wq, wk, wv, wproj):
    cos = np.asarray(cos, np.float32)
    sin = np.asarray(sin, np.float32)
    cc = np.tile(np.concatenate([cos, cos], axis=1), (1, NH))      # [T, NH*D]
    ss = np.tile(np.concatenate([sin, -sin], axis=1), (1, NH))     # [T, NH*D]
    # partition-major images: contiguous per-partition DMA lines
    cch = np.ascontiguousarray(cc.reshape(NT, 128, NH * D).transpose(1, 0, 2))
    ssh = np.ascontiguousarray(ss.reshape(NT, 128, NH * D).transpose(1, 0, 2))
    ident = np.eye(128, dtype=np.float32)
    # msk[p, u] = 1 iff u >= p + 384; diagonal block k uses cols [384-128k, ...)
    p = np.arange(128)[:, None]
    u = np.arange(384 + QC)[None, :]
    mask = (u >= p + 384).astype(np.float32)

    x = np.asarray(x, np.float32)
    wq = np.asarray(wq, np.float32)
    wk = np.asarray(wk, np.float32)
    wv = np.asarray(wv, np.float32)
    wpT = np.asarray(wproj, np.float32).T

    maps = []
    for c in range(8):
        b = c // 2
        hs = (c % 2) * NH
        sl = slice(hs * D, (hs + NH) * D)
        # xh[p, tt, ci, u] = x[b][tt*128+u, ci*128+p]
        xhost = x[b].reshape(NT, 128, CT, 128).transpose(3, 0, 2, 1)
        # wqh[p, ci, o] = wq[sl][o, ci*128+p]
        wqhost = wq[sl].T.reshape(CT, 128, NH * D).transpose(1, 0, 2)
        wkhost = wk[sl].T.reshape(CT, 128, NH * D).transpose(1, 0, 2)
        wvhost = wv[sl].T.reshape(CT, 128, NH * D).transpose(1, 0, 2)
        # wph[p, h, c] = wproj.T[sl][h*128+p, c]
        wphost = wpT[sl].reshape(NH, 128, C).transpose(1, 0, 2)
        maps.append({
            "xh": _bf16(xhost),
            "wqh": _bf16(wqhost),
            "wkh": _bf16(wkhost),
            "wvh": _bf16(wvhost),
            "wph": _bf16(wphost),
            "cch": _bf16(cch),
            "ssh": _bf16(ssh),
            "ident": _bf16(ident),
            "ones_in": _bf16(np.ones((128, 1), dtype=np.float32)),
            "mask_in": _bf16(mask),
        })
    return maps


def kernel(x, cos, sin, wq, wk, wv, wproj, _trace=False):
    nc = _get_nc()
    maps = _in_maps(x, cos, sin, wq, wk, wv, wproj)
    res = run_bass_kernel_spmd(nc, maps, core_ids=list(range(8)), trace=_trace)
    parts = [np.asarray(r["out"], dtype=np.float32) for r in res.results]
    outv = np.stack([parts[2 * b] + parts[2 * b + 1] for b in range(B)])
    if _trace:
        _CACHE["last_results"] = res
    return outv.astype(np.float32)


# revision 9
# speedup vs baseline: 1.2570x; 1.2394x over previous
"""Causal self-attention (B=4, T=2048, C=768, H=6, D=128) on 8 trn2 NeuronCores.

Sharding: 24 (batch, head) units -> 8 cores, each core owns 1 batch x 3 heads.
Per core: QKV projections for its 3 heads, RoPE + per-head norm, causal
attention, partial output projection over its heads' columns.
Unshard: out[b] = partial[core 2b] + partial[core 2b+1]  (tensor-parallel sum).

v5 (pipelining rework of v4; the v4 trace showed stage 3 waiting until t=142us
because the stage-1/stage-3 PSUM pools could not coexist):
  - every matmul operand is bf16 (1 cyc/col streaming + fast weight load);
    fp32 PSUM accumulation. rel-err vs fp32 reference ~6e-3, gate 2e-2.
  - host-swizzled partition-major input images (contiguous DMA lines).
  - loop A: QKV matmuls + rope (4 bf16 DVE ops via CC=[cos|cos]/SS=[sin|-sin]
    tables) + per-head bn_stats, all 16 token tiles.
  - ONE batched stats combine: rstd = exp(-0.5*ln(var128/127)) on [128, 96]
    tiles - Ln and Exp load their ACT tables once (no per-tile thrash);
    eps dropped (std ~0.55, effect 2e-6).
  - loop B (norm-apply + PE transpose, per 4-tile group) is INTERLEAVED with
    attention chunks: group g unblocks chunk g; all PSUM pools (3x single
    score blocks, 2x yps, dps, proj, transposes) co-fit in the 16KB budget so
    the Tile scheduler overlaps stage-1 tails with attention.
  - norm-apply split across DVE (tensor_scalar) and ACT (Identity bias/scale)
    to balance engines; same for the PSUM->SBUF copies.
  - causal mask via one host [128, 896] 0/1 bf16 sliding-window table: one
    DVE multiply per diagonal block.
  - softmax denominator: ones^T @ et on the PE; reciprocal_approx_fast +
    gpsimd partition_broadcast for the normalize.
  - partial outputs leave in bf16; host sums core pairs in fp32.
"""

import ml_dtypes
import numpy as np

import concourse.bacc as bacc
import concourse.bass as bass
import concourse.mybir as mybir
from concourse import tile
from concourse.bass_utils import run_bass_kernel_spmd

F32 = mybir.dt.float32
BF16 = mybir.dt.bfloat16
AF = mybir.ActivationFunctionType
ALU = mybir.AluOpType

B, T, C, H, D = 4, 2048, 768, 6, 128
HALF = D // 2
NH = 3            # heads per core
CT = C // 128     # 6 contraction tiles for projections
NT = T // 128     # 16 token tiles
QC = 512          # query-chunk width for attention
NQC = T // QC     # 4 chunks
SCALE = 1.0 / float(np.sqrt(D))

_CACHE = {}


def _build_nc():
    nc = bacc.Bacc("TRN2")

    xh = nc.dram_tensor("xh", [128, NT, CT, 128], BF16, kind="ExternalInput")
    wqh = nc.dram_tensor("wqh", [128, CT, NH * D], BF16, kind="ExternalInput")
    wkh = nc.dram_tensor("wkh", [128, CT, NH * D], BF16, kind="ExternalInput")
    wvh = nc.dram_tensor("wvh", [128, CT, NH * D], BF16, kind="ExternalInput")
    wph = nc.dram_tensor("wph", [128, NH, C], BF16, kind="ExternalInput")
    cch = nc.dram_tensor("cch", [128, NT, NH * D], BF16, kind="ExternalInput")
    ssh = nc.dram_tensor("ssh", [128, NT, NH * D], BF16, kind="ExternalInput")
    ident = nc.dram_tensor("ident", [128, 128], BF16, kind="ExternalInput")
    ones_in = nc.dram_tensor("ones_in", [128, 1], BF16, kind="ExternalInput")
    mask_in = nc.dram_tensor("mask_in", [128, 384 + QC], BF16, kind="ExternalInput")
    out = nc.dram_tensor("out", [T, C], BF16, kind="ExternalOutput")

    with tile.TileContext(nc) as tc:
        with (
            tc.tile_pool(name="persist", bufs=1) as persist,
            tc.tile_pool(name="qkvbuf", bufs=1) as qkvbuf,
            tc.tile_pool(name="stat", bufs=1) as spool,
        ):
            QT = qkvbuf.tile([128, NH, T], BF16)       # [d, h, t]
            KT = qkvbuf.tile([128, NH, T], BF16)       # [d, h, t]
            V = qkvbuf.tile([128, NT, NH * D], BF16)   # [s%128, s//128, h*D+d]
            RQ = qkvbuf.tile([128, NT, NH * D], BF16)  # rope(q), pre-norm
            RK = qkvbuf.tile([128, NT, NH * D], BF16)
            ones = persist.tile([128, 1], BF16)
            idn = persist.tile([128, 128], BF16)
            wp_sb = persist.tile([128, NH, C], BF16)   # [d, h, c]
            msk = persist.tile([128, 384 + QC], BF16)
            # stats for all tiles: [tile, q/k, head, bn6]
            Sall = spool.tile([128, NT, 2, NH, 6], F32)
            G = NT * 2 * NH
            rstd = spool.tile([128, G], F32)
            nmrs = spool.tile([128, G], F32)

            # ---------------- stage 1: QKV projection + rope + bn_stats -------
            with (
                tc.tile_pool(name="wbuf", bufs=1) as wbuf,
                tc.tile_pool(name="xch", bufs=3) as xpool,
                tc.tile_pool(name="rope", bufs=4) as rpool,
                tc.tile_pool(name="psA", bufs=3, space="PSUM") as psA,
            ):
                wq_sb = wbuf.tile([128, CT, NH * D], BF16)
                wk_sb = wbuf.tile([128, CT, NH * D], BF16)
                wv_sb = wbuf.tile([128, CT, NH * D], BF16)
                # startup-latency ordering: first-tile deps first
                nc.sync.dma_start(wq_sb[:, 0:3], wqh[:, 0:3])
                nc.sync.dma_start(wk_sb[:, 0:3], wkh[:, 0:3])
                nc.sync.dma_start(wv_sb[:, 0:3], wvh[:, 0:3])
                xch0 = xpool.tile([128, CT, 128], BF16, tag="xch")
                nc.sync.dma_start(xch0[:], xh[:, 0])
                nc.sync.dma_start(wq_sb[:, 3:CT], wqh[:, 3:CT])
                nc.sync.dma_start(wk_sb[:, 3:CT], wkh[:, 3:CT])
                nc.sync.dma_start(wv_sb[:, 3:CT], wvh[:, 3:CT])

                cc_sb = wbuf.tile([128, NT, NH * D], BF16)
                ss_sb = wbuf.tile([128, NT, NH * D], BF16)
                nc.sync.dma_start(cc_sb[:], cch[:])
                nc.sync.dma_start(ss_sb[:], ssh[:])
                nc.sync.dma_start(idn[:], ident[:])
                nc.sync.dma_start(wp_sb[:], wph[:])
                nc.sync.dma_start(ones[:], ones_in[:])
                nc.sync.dma_start(msk[:], mask_in[:])

                # ---- loop A: projections + rope + bn_stats -------------------
                for tt in range(NT):
                    if tt == 0:
                        xch = xch0
                    else:
                        xch = xpool.tile([128, CT, 128], BF16, tag="xch")
                        nc.sync.dma_start(xch[:], xh[:, tt])

                    qps = psA.tile([128, NH * D], F32, tag="ps")
                    kps = psA.tile([128, NH * D], F32, tag="ps")
                    vps = psA.tile([128, NH * D], F32, tag="ps")
                    for ci in range(CT):
                        st_, sp_ = (ci == 0), (ci == CT - 1)
                        lhs = xch[:, ci, :]
                        nc.tensor.matmul(qps[:], lhs, wq_sb[:, ci, :], start=st_, stop=sp_)
                        nc.tensor.matmul(kps[:], lhs, wk_sb[:, ci, :], start=st_, stop=sp_)
                        nc.tensor.matmul(vps[:], lhs, wv_sb[:, ci, :], start=st_, stop=sp_)

                    # V: straight copy PSUM -> SBUF bf16 in natural [t, o] layout
                    nc.scalar.copy(V[:, tt, :], vps[:])

                    for mi, ps, R in ((0, qps, RQ), (1, kps, RK)):
                        sb = rpool.tile([128, NH * D], BF16, tag=f"sb{mi}")
                        nc.scalar.copy(sb[:], ps[:])
                        sb_v = sb[:].rearrange("p (h d) -> p h d", h=NH)
                        ss_v = ss_sb[:, tt].rearrange("p (h d) -> p h d", h=NH)
                        r = R[:, tt, :]
                        r_v = r.rearrange("p (h d) -> p h d", h=NH)
                        t2 = rpool.tile([128, NH * D], BF16, tag=f"t2{mi}")
                        t2_v = t2[:].rearrange("p (h d) -> p h d", h=NH)
                        # rope: r = u*CC + swap(u)*SS, swap done by half-slices
                        nc.vector.tensor_mul(
                            t2_v[:, :, 0:HALF], sb_v[:, :, HALF:D], ss_v[:, :, 0:HALF])
                        nc.vector.tensor_mul(
                            t2_v[:, :, HALF:D], sb_v[:, :, 0:HALF], ss_v[:, :, HALF:D])
                        nc.vector.tensor_mul(r, sb[:], cc_sb[:, tt, :])
                        nc.vector.tensor_add(r, r, t2[:])
                        for h in range(NH):
                            nc.vector.bn_stats(Sall[:, tt, mi, h], r_v[:, h])

                # ---- one combine for all tiles: rstd/nmrs --------------------
                # var*128 = cv_e + cv_o + 32*(m_e - m_o)^2   (ddof=1 -> /127)
                dm = rpool.tile([128, G], F32, tag="dm")
                ms = rpool.tile([128, G], F32, tag="ms")
                cv = rpool.tile([128, G], F32, tag="cv")
                s2 = rpool.tile([128, G], F32, tag="s2")
                var = rpool.tile([128, G], F32, tag="var")
                lnv = rpool.tile([128, G], F32, tag="lnv")
                m_e = Sall[:, :, :, :, 1]
                m_o = Sall[:, :, :, :, 4]
                cv_e = Sall[:, :, :, :, 2]
                cv_o = Sall[:, :, :, :, 5]
                sh = dict(a=NT, b=2)
                dm_v = dm[:].rearrange("p (a b c) -> p a b c", **sh)
                ms_v = ms[:].rearrange("p (a b c) -> p a b c", **sh)
                cv_v = cv[:].rearrange("p (a b c) -> p a b c", **sh)
                nc.vector.tensor_sub(dm_v, m_e, m_o)
                nc.vector.tensor_add(ms_v, m_e, m_o)
                nc.vector.tensor_add(cv_v, cv_e, cv_o)
                nc.vector.scalar_tensor_tensor(
                    s2[:], dm[:], 32.0, dm[:], op0=ALU.mult, op1=ALU.mult)
                nc.vector.tensor_add(var[:], cv[:], s2[:])
                nc.scalar.activation(lnv[:], var[:], AF.Ln, scale=1.0 / (D - 1))
                nc.scalar.activation(rstd[:], lnv[:], AF.Exp, scale=-0.5)
                nc.vector.scalar_tensor_tensor(
                    nmrs[:], ms[:], -0.5, rstd[:], op0=ALU.mult, op1=ALU.mult)

            rstd_v = rstd[:].rearrange("p (a b c) -> p a b c", a=NT, b=2)
            nmrs_v = nmrs[:].rearrange("p (a b c) -> p a b c", a=NT, b=2)

            # ------- stage 2+3+4 interleaved: norm+transpose | attention ------
            with (
                tc.tile_pool(name="nbuf", bufs=4) as npool,
                tc.tile_pool(name="att", bufs=3) as apool,
                tc.tile_pool(name="acc", bufs=2) as accpool,
                tc.tile_pool(name="ybuf", bufs=2) as ypool,
                tc.tile_pool(name="obuf", bufs=3) as opool,
                tc.tile_pool(name="psT", bufs=1, space="PSUM") as psT,
                tc.tile_pool(name="psS", bufs=3, space="PSUM") as psS,
                tc.tile_pool(name="psY", bufs=2, space="PSUM") as psY,
                tc.tile_pool(name="psD", bufs=1, space="PSUM") as psD,
                tc.tile_pool(name="psP", bufs=1, space="PSUM") as psP,
            ):
                out_r = out.rearrange("(tt p) c -> p tt c", p=128)

                def norm_transpose(tt):
                    for mi, R, dstT in ((0, RQ, QT), (1, RK, KT)):
                        nrm = npool.tile([128, NH * D], BF16, tag=f"n{mi}")
                        tps = psT.tile([128, NH * D], BF16, tag="tp")
                        for h in range(NH):
                            rsl = rstd_v[:, tt, mi, h:h + 1]
                            nsl = nmrs_v[:, tt, mi, h:h + 1]
                            dsl = slice(h * D, (h + 1) * D)
                            if mi == 0:
                                nc.vector.tensor_scalar(
                                    nrm[:, dsl], R[:, tt, dsl], rsl, nsl,
                                    ALU.mult, ALU.add)
                            else:
                                # same affine on the ACT engine (balance DVE)
                                nc.scalar.activation(
                                    nrm[:, dsl], R[:, tt, dsl], AF.Identity,
                                    bias=nsl, scale=rsl)
                            nc.tensor.transpose(tps[:, dsl], nrm[:, dsl], idn[:])
                        dst = dstT[:, :, tt * 128:(tt + 1) * 128]
                        src = tps[:].rearrange("p (h t) -> p h t", h=NH)
                        if mi == 0:
                            nc.scalar.copy(dst, src)
                        else:
                            nc.vector.tensor_copy(dst, src)

                def emit_proj(qc, yTc):
                    # output projection for chunk qc's 4 token tiles
                    for j in range(QC // 128):
                        tt = qc * (QC // 128) + j
                        ot = opool.tile([128, C], BF16, tag="ot")
                        for half in range(2):
                            op_ps = psP.tile([128, C // 2], F32, tag="pp")
                            csl = slice(half * (C // 2), (half + 1) * (C // 2))
                            for h in range(NH):
                                lhs = yTc[:, h, j * 128:(j + 1) * 128]
                                nc.tensor.matmul(op_ps[:], lhs, wp_sb[:, h, csl],
                                                 start=(h == 0), stop=(h == NH - 1))
                            nc.vector.tensor_copy(ot[:, csl], op_ps[:])
                        nc.sync.dma_start(out_r[:, tt, :], ot[:])

                pending = None
                for qc in range(NQC):
                    # unblock this chunk: norm+transpose its 4 token tiles
                    for j in range(QC // 128):
                        norm_transpose(qc * (QC // 128) + j)

                    Q0 = qc * QC
                    n_st = (Q0 + QC) // 128
                    yTc = ypool.tile([128, NH, QC], BF16, tag="yT")  # [d, h, q]
                    for h in range(NH):
                        yps = psY.tile([128, QC], F32, tag="yps")
                        dps = psD.tile([1, QC], F32, tag="dps")
                        for st in range(n_st):
                            sps = psS.tile([128, QC], F32, tag="sps")
                            nc.tensor.matmul(
                                sps[:],
                                KT[:, h, st * 128:(st + 1) * 128],
                                QT[:, h, Q0:Q0 + QC],
                                start=True, stop=True,
                            )
                            et = apool.tile([128, QC], BF16, tag="et")
                            nc.scalar.activation(et[:], sps[:], AF.Exp, scale=SCALE)
                            k = st - n_st + 4
                            if k >= 0:  # diagonal block: zero where s > q
                                nc.vector.tensor_mul(
                                    et[:], et[:], msk[:, 384 - 128 * k:896 - 128 * k])
                            nc.tensor.matmul(
                                yps[:],
                                V[:, st, h * D:(h + 1) * D],
                                et[:],
                                start=(st == 0), stop=(st == n_st - 1),
                                skip_group_check=True,
                            )
                            # softmax denominator on PE: ones^T @ exp
                            nc.tensor.matmul(
                                dps[:1, :],
                                ones[:],
                                et[:],
                                start=(st == 0), stop=(st == n_st - 1),
                                skip_group_check=True,
                            )
                        rc1 = accpool.tile([1, QC], F32, tag="rc1")
                        nc.vector.reciprocal_approx_fast(rc1[:1, :], dps[:1, :])
                        rbc = accpool.tile([128, QC], F32, tag="rbc")
                        nc.gpsimd.partition_broadcast(rbc[:], rc1[:1, :])
                        nc.vector.tensor_mul(yTc[:, h, :], yps[:], rbc[:])
                        if h == 0 and pending is not None:
                            # previous chunk's projection lands here so its
                            # yTc-normalize latency hides under this chunk's
                            # independent attention matmuls
                            emit_proj(*pending)
                            pending = None

                    pending = (qc, yTc)
                emit_proj(*pending)

    nc.compile()
    return nc


def _get_nc():
    if "nc" not in _CACHE:
        _CACHE["nc"] = _build_nc()
    return _CACHE["nc"]


def _bf16(a):
    return np.ascontiguousarray(np.asarray(a, np.float32)).astype(ml_dtypes.bfloat16)


def _in_maps(x, cos, sin, wq, wk, wv, wproj):
    cos = np.asarray(cos, np.float32)
    sin = np.asarray(sin, np.float32)
    cc = np.tile(np.concatenate([cos, cos], axis=1), (1, NH))      # [T, NH*D]
    ss = np.tile(np.concatenate([sin, -sin], axis=1), (1, NH))     # [T, NH*D]
    # partition-major images: contiguous per-partition DMA lines
    cchost = np.ascontiguousarray(cc.reshape(NT, 128, NH * D).transpose(1, 0, 2))
    sshost = np.ascontiguousarray(ss.reshape(NT, 128, NH * D).transpose(1, 0, 2))
    ident = np.eye(128, dtype=np.float32)
    # msk[p, u] = 1 iff u >= p + 384; diagonal block k uses cols [384-128k, ...)
    p = np.arange(128)[:, None]
    u = np.arange(384 + QC)[None, :]
    mask = (u >= p + 384).astype(np.float32)

    x = np.asarray(x, np.float32)
    wq = np.asarray(wq, np.float32)
    wk = np.asarray(wk, np.float32)
    wv = np.asarray(wv, np.float32)
    wpT = np.asarray(wproj, np.float32).T

    maps = []
    for c in range(8):
        b = c // 2
        hs = (c % 2) * NH
        sl = slice(hs * D, (hs + NH) * D)
        # xh[p, tt, ci, u] = x[b][tt*128+u, ci*128+p]
        xhost = x[b].reshape(NT, 128, CT, 128).transpose(3, 0, 2, 1)
        # wqh[p, ci, o] = wq[sl][o, ci*128+p]
        wqhost = wq[sl].T.reshape(CT, 128, NH * D).transpose(1, 0, 2)
        wkhost = wk[sl].T.reshape(CT, 128, NH * D).transpose(1, 0, 2)
        wvhost = wv[sl].T.reshape(CT, 128, NH * D).transpose(1, 0, 2)
        # wph[p, h, c] = wproj.T[sl][h*128+p, c]
        wphost = wpT[sl].reshape(NH, 128, C).transpose(1, 0, 2)
        maps.append({
            "xh": _bf16(xhost),
            "wqh": _bf16(wqhost),
            "wkh": _bf16(wkhost),
            "wvh": _bf16(wvhost),
            "wph": _bf16(wphost),
            "cch": _bf16(cchost),
            "ssh": _bf16(sshost),
            "ident": _bf16(ident),
            "ones_in": _bf16(np.ones((128, 1), dtype=np.float32)),
            "mask_in": _bf16(mask),
        })
    return maps


def kernel(x, cos, sin, wq, wk, wv, wproj, _trace=False):
    nc = _get_nc()
    maps = _in_maps(x, cos, sin, wq, wk, wv, wproj)
    res = run_bass_kernel_spmd(nc, maps, core_ids=list(range(8)), trace=_trace)
    parts = [np.asarray(r["out"], dtype=np.float32) for r in res.results]
    outv = np.stack([parts[2 * b] + parts[2 * b + 1] for b in range(B)])
    if _trace:
        _CACHE["last_results"] = res
    return outv.astype(np.float32)


# revision 10
# speedup vs baseline: 1.2773x; 1.0161x over previous
"""Causal self-attention (B=4, T=2048, C=768, H=6, D=128) on 8 trn2 NeuronCores.

Sharding: 24 (batch, head) units -> 8 cores, each core owns 1 batch x 3 heads.
Per core: QKV projections for its 3 heads, RoPE + per-head norm, causal
attention, partial output projection over its heads' columns.
Unshard: out[b] = partial[core 2b] + partial[core 2b+1]  (tensor-parallel sum).

v5 (pipelining rework of v4; the v4 trace showed stage 3 waiting until t=142us
because the stage-1/stage-3 PSUM pools could not coexist):
  - every matmul operand is bf16 (1 cyc/col streaming + fast weight load);
    fp32 PSUM accumulation. rel-err vs fp32 reference ~6e-3, gate 2e-2.
  - host-swizzled partition-major input images (contiguous DMA lines).
  - loop A: QKV matmuls + rope (4 bf16 DVE ops via CC=[cos|cos]/SS=[sin|-sin]
    tables) + per-head bn_stats, all 16 token tiles.
  - ONE batched stats combine: rstd = exp(-0.5*ln(var128/127)) on [128, 96]
    tiles - Ln and Exp load their ACT tables once (no per-tile thrash);
    eps dropped (std ~0.55, effect 2e-6).
  - loop B (norm-apply + PE transpose, per 4-tile group) is INTERLEAVED with
    attention chunks: group g unblocks chunk g; all PSUM pools (3x single
    score blocks, 2x yps, dps, proj, transposes) co-fit in the 16KB budget so
    the Tile scheduler overlaps stage-1 tails with attention.
  - norm-apply split across DVE (tensor_scalar) and ACT (Identity bias/scale)
    to balance engines; same for the PSUM->SBUF copies.
  - causal mask via one host [128, 896] 0/1 bf16 sliding-window table: one
    DVE multiply per diagonal block.
  - softmax denominator: ones^T @ et on the PE; reciprocal_approx_fast +
    gpsimd partition_broadcast for the normalize.
  - partial outputs leave in bf16; host sums core pairs in fp32.
"""

import ml_dtypes
import numpy as np

import concourse.bacc as bacc
import concourse.bass as bass
import concourse.mybir as mybir
from concourse import tile
from concourse.bass_utils import run_bass_kernel_spmd

F32 = mybir.dt.float32
BF16 = mybir.dt.bfloat16
AF = mybir.ActivationFunctionType
ALU = mybir.AluOpType

B, T, C, H, D = 4, 2048, 768, 6, 128
HALF = D // 2
NH = 3            # heads per core
CT = C // 128     # 6 contraction tiles for projections
NT = T // 128     # 16 token tiles
QC = 512          # query-chunk width for attention
NQC = T // QC     # 4 chunks
SCALE = 1.0 / float(np.sqrt(D))

_CACHE = {}


def _build_nc():
    nc = bacc.Bacc("TRN2")

    xh = nc.dram_tensor("xh", [128, NT, CT, 128], BF16, kind="ExternalInput")
    wqh = nc.dram_tensor("wqh", [128, CT, NH * D], BF16, kind="ExternalInput")
    wkh = nc.dram_tensor("wkh", [128, CT, NH * D], BF16, kind="ExternalInput")
    wvh = nc.dram_tensor("wvh", [128, CT, NH * D], BF16, kind="ExternalInput")
    wph = nc.dram_tensor("wph", [128, NH, C], BF16, kind="ExternalInput")
    cch = nc.dram_tensor("cch", [128, NT, NH * D], BF16, kind="ExternalInput")
    ssh = nc.dram_tensor("ssh", [128, NT, NH * D], BF16, kind="ExternalInput")
    ident = nc.dram_tensor("ident", [128, 128], BF16, kind="ExternalInput")
    ones_in = nc.dram_tensor("ones_in", [128, 1], BF16, kind="ExternalInput")
    mask_in = nc.dram_tensor("mask_in", [128, 384 + QC], BF16, kind="ExternalInput")
    out = nc.dram_tensor("out", [T, C], BF16, kind="ExternalOutput")

    with tile.TileContext(nc) as tc:
        with (
            tc.tile_pool(name="persist", bufs=1) as persist,
            tc.tile_pool(name="qkvbuf", bufs=1) as qkvbuf,
            tc.tile_pool(name="stat", bufs=1) as spool,
        ):
            QT = qkvbuf.tile([128, NH, T], BF16)       # [d, h, t]
            KT = qkvbuf.tile([128, NH, T], BF16)       # [d, h, t]
            V = qkvbuf.tile([128, NT, NH * D], BF16)   # [s%128, s//128, h*D+d]
            RQ = qkvbuf.tile([128, NT, NH * D], BF16)  # rope(q), pre-norm
            RK = qkvbuf.tile([128, NT, NH * D], BF16)
            ones = persist.tile([128, 1], BF16)
            idn = persist.tile([128, 128], BF16)
            wp_sb = persist.tile([128, NH, C], BF16)   # [d, h, c]
            msk = persist.tile([128, 384 + QC], BF16)
            # stats for all tiles: [tile, q/k, head, bn6]
            Sall = spool.tile([128, NT, 2, NH, 6], F32)
            G = NT * 2 * NH
            rstd = spool.tile([128, G], F32)
            nmrs = spool.tile([128, G], F32)

            # ---------------- stage 1: QKV projection + rope + bn_stats -------
            with (
                tc.tile_pool(name="wbuf", bufs=1) as wbuf,
                tc.tile_pool(name="xch", bufs=3) as xpool,
                tc.tile_pool(name="rope", bufs=4) as rpool,
                tc.tile_pool(name="psA", bufs=3, space="PSUM") as psA,
            ):
                wq_sb = wbuf.tile([128, CT, NH * D], BF16)
                wk_sb = wbuf.tile([128, CT, NH * D], BF16)
                wv_sb = wbuf.tile([128, CT, NH * D], BF16)
                # startup-latency ordering: first-tile deps first
                nc.sync.dma_start(wq_sb[:, 0:3], wqh[:, 0:3])
                nc.sync.dma_start(wk_sb[:, 0:3], wkh[:, 0:3])
                nc.sync.dma_start(wv_sb[:, 0:3], wvh[:, 0:3])
                xch0 = xpool.tile([128, CT, 128], BF16, tag="xch")
                nc.sync.dma_start(xch0[:], xh[:, 0])
                nc.sync.dma_start(wq_sb[:, 3:CT], wqh[:, 3:CT])
                nc.sync.dma_start(wk_sb[:, 3:CT], wkh[:, 3:CT])
                nc.sync.dma_start(wv_sb[:, 3:CT], wvh[:, 3:CT])

                cc_sb = wbuf.tile([128, NT, NH * D], BF16)
                ss_sb = wbuf.tile([128, NT, NH * D], BF16)
                nc.scalar.dma_start(cc_sb[:], cch[:])
                nc.scalar.dma_start(ss_sb[:], ssh[:])
                nc.scalar.dma_start(idn[:], ident[:])
                nc.scalar.dma_start(wp_sb[:], wph[:])
                nc.scalar.dma_start(ones[:], ones_in[:])
                nc.scalar.dma_start(msk[:], mask_in[:])

                def combine(t0, t1):
                    # rstd/nmrs for token tiles [t0, t1):
                    # var*128 = cv_e + cv_o + 32*(m_e - m_o)^2  (ddof=1 -> /127)
                    n = t1 - t0
                    g0, g1 = t0 * 2 * NH, t1 * 2 * NH
                    gw = g1 - g0
                    dm = rpool.tile([128, gw], F32, tag="dm")
                    ms = rpool.tile([128, gw], F32, tag="ms")
                    cv = rpool.tile([128, gw], F32, tag="cv")
                    s2 = rpool.tile([128, gw], F32, tag="s2")
                    var = rpool.tile([128, gw], F32, tag="var")
                    lnv = rpool.tile([128, gw], F32, tag="lnv")
                    S = Sall[:, t0:t1]
                    m_e, m_o = S[:, :, :, :, 1], S[:, :, :, :, 4]
                    cv_e, cv_o = S[:, :, :, :, 2], S[:, :, :, :, 5]
                    sh = dict(a=n, b=2)
                    nc.vector.tensor_sub(
                        dm[:].rearrange("p (a b c) -> p a b c", **sh), m_e, m_o)
                    nc.vector.tensor_add(
                        ms[:].rearrange("p (a b c) -> p a b c", **sh), m_e, m_o)
                    nc.vector.tensor_add(
                        cv[:].rearrange("p (a b c) -> p a b c", **sh), cv_e, cv_o)
                    nc.vector.scalar_tensor_tensor(
                        s2[:], dm[:], 32.0, dm[:], op0=ALU.mult, op1=ALU.mult)
                    nc.vector.tensor_add(var[:], cv[:], s2[:])
                    nc.scalar.activation(lnv[:], var[:], AF.Ln, scale=1.0 / (D - 1))
                    nc.scalar.activation(rstd[:, g0:g1], lnv[:], AF.Exp, scale=-0.5)
                    nc.vector.scalar_tensor_tensor(
                        nmrs[:, g0:g1], ms[:], -0.5, rstd[:, g0:g1],
                        op0=ALU.mult, op1=ALU.mult)

                # ---- loop A: projections + rope + bn_stats -------------------
                for tt in range(NT):
                    if tt == 0:
                        xch = xch0
                    else:
                        xch = xpool.tile([128, CT, 128], BF16, tag="xch")
                        nc.sync.dma_start(xch[:], xh[:, tt])

                    qps = psA.tile([128, NH * D], F32, tag="ps")
                    kps = psA.tile([128, NH * D], F32, tag="ps")
                    vps = psA.tile([128, NH * D], F32, tag="ps")
                    for ci in range(CT):
                        st_, sp_ = (ci == 0), (ci == CT - 1)
                        lhs = xch[:, ci, :]
                        nc.tensor.matmul(qps[:], lhs, wq_sb[:, ci, :], start=st_, stop=sp_)
                        nc.tensor.matmul(kps[:], lhs, wk_sb[:, ci, :], start=st_, stop=sp_)
                        nc.tensor.matmul(vps[:], lhs, wv_sb[:, ci, :], start=st_, stop=sp_)

                    # V: straight copy PSUM -> SBUF bf16 in natural [t, o] layout
                    nc.scalar.copy(V[:, tt, :], vps[:])

                    for mi, ps, R in ((0, qps, RQ), (1, kps, RK)):
                        sb = rpool.tile([128, NH * D], BF16, tag=f"sb{mi}")
                        nc.scalar.copy(sb[:], ps[:])
                        sb_v = sb[:].rearrange("p (h d) -> p h d", h=NH)
                        ss_v = ss_sb[:, tt].rearrange("p (h d) -> p h d", h=NH)
                        r = R[:, tt, :]
                        r_v = r.rearrange("p (h d) -> p h d", h=NH)
                        t2 = rpool.tile([128, NH * D], BF16, tag=f"t2{mi}")
                        t2_v = t2[:].rearrange("p (h d) -> p h d", h=NH)
                        # rope: r = u*CC + swap(u)*SS, swap done by half-slices
                        nc.vector.tensor_mul(
                            t2_v[:, :, 0:HALF], sb_v[:, :, HALF:D], ss_v[:, :, 0:HALF])
                        nc.vector.tensor_mul(
                            t2_v[:, :, HALF:D], sb_v[:, :, 0:HALF], ss_v[:, :, HALF:D])
                        nc.vector.tensor_mul(r, sb[:], cc_sb[:, tt, :])
                        nc.vector.tensor_add(r, r, t2[:])
                        for h in range(NH):
                            nc.vector.bn_stats(Sall[:, tt, mi, h], r_v[:, h])
                    if tt == 3:
                        combine(0, 4)   # unblock chunk-0 norm+transpose early
                    elif tt == NT - 1:
                        combine(4, NT)


            rstd_v = rstd[:].rearrange("p (a b c) -> p a b c", a=NT, b=2)
            nmrs_v = nmrs[:].rearrange("p (a b c) -> p a b c", a=NT, b=2)

            # ------- stage 2+3+4 interleaved: norm+transpose | attention ------
            with (
                tc.tile_pool(name="nbuf", bufs=4) as npool,
                tc.tile_pool(name="att", bufs=3) as apool,
                tc.tile_pool(name="acc", bufs=2) as accpool,
                tc.tile_pool(name="ybuf", bufs=2) as ypool,
                tc.tile_pool(name="obuf", bufs=3) as opool,
                tc.tile_pool(name="psT", bufs=1, space="PSUM") as psT,
                tc.tile_pool(name="psS", bufs=3, space="PSUM") as psS,
                tc.tile_pool(name="psY", bufs=2, space="PSUM") as psY,
                tc.tile_pool(name="psD", bufs=1, space="PSUM") as psD,
                tc.tile_pool(name="psP", bufs=1, space="PSUM") as psP,
            ):
                out_r = out.rearrange("(tt p) c -> p tt c", p=128)

                def norm_transpose(tt):
                    for mi, R, dstT in ((0, RQ, QT), (1, RK, KT)):
                        nrm = npool.tile([128, NH * D], BF16, tag=f"n{mi}")
                        tps = psT.tile([128, NH * D], BF16, tag="tp")
                        for h in range(NH):
                            rsl = rstd_v[:, tt, mi, h:h + 1]
                            nsl = nmrs_v[:, tt, mi, h:h + 1]
                            dsl = slice(h * D, (h + 1) * D)
                            if mi == 0:
                                nc.vector.tensor_scalar(
                                    nrm[:, dsl], R[:, tt, dsl], rsl, nsl,
                                    ALU.mult, ALU.add)
                            else:
                                # same affine on the ACT engine (balance DVE)
                                nc.scalar.activation(
                                    nrm[:, dsl], R[:, tt, dsl], AF.Identity,
                                    bias=nsl, scale=rsl)
                            nc.tensor.transpose(tps[:, dsl], nrm[:, dsl], idn[:])
                        dst = dstT[:, :, tt * 128:(tt + 1) * 128]
                        src = tps[:].rearrange("p (h t) -> p h t", h=NH)
                        if mi == 0:
                            nc.scalar.copy(dst, src)
                        else:
                            nc.vector.tensor_copy(dst, src)

                def emit_proj(qc, yTc):
                    # output projection for chunk qc's 4 token tiles
                    for j in range(QC // 128):
                        tt = qc * (QC // 128) + j
                        ot = opool.tile([128, C], BF16, tag="ot")
                        for half in range(2):
                            op_ps = psP.tile([128, C // 2], F32, tag="pp")
                            csl = slice(half * (C // 2), (half + 1) * (C // 2))
                            for h in range(NH):
                                lhs = yTc[:, h, j * 128:(j + 1) * 128]
                                nc.tensor.matmul(op_ps[:], lhs, wp_sb[:, h, csl],
                                                 start=(h == 0), stop=(h == NH - 1))
                            nc.vector.tensor_copy(ot[:, csl], op_ps[:])
                        nc.sync.dma_start(out_r[:, tt, :], ot[:])

                pending = None
                for qc in range(NQC):
                    # unblock this chunk: norm+transpose its 4 token tiles
                    for j in range(QC // 128):
                        norm_transpose(qc * (QC // 128) + j)

                    Q0 = qc * QC
                    n_st = (Q0 + QC) // 128
                    yTc = ypool.tile([128, NH, QC], BF16, tag="yT")  # [d, h, q]
                    for h in range(NH):
                        yps = psY.tile([128, QC], F32, tag="yps")
                        dps = psD.tile([1, QC], F32, tag="dps")
                        for st in range(n_st):
                            sps = psS.tile([128, QC], F32, tag="sps")
                            nc.tensor.matmul(
                                sps[:],
                                KT[:, h, st * 128:(st + 1) * 128],
                                QT[:, h, Q0:Q0 + QC],
                                start=True, stop=True,
                            )
                            et = apool.tile([128, QC], BF16, tag="et")
                            nc.scalar.activation(et[:], sps[:], AF.Exp, scale=SCALE)
                            k = st - n_st + 4
                            if k >= 0:  # diagonal block: zero where s > q
                                nc.vector.tensor_mul(
                                    et[:], et[:], msk[:, 384 - 128 * k:896 - 128 * k])
                            nc.tensor.matmul(
                                yps[:],
                                V[:, st, h * D:(h + 1) * D],
                                et[:],
                                start=(st == 0), stop=(st == n_st - 1),
                                skip_group_check=True,
                            )
                            # softmax denominator on PE: ones^T @ exp
                            nc.tensor.matmul(
                                dps[:1, :],
                                ones[:],
                                et[:],
                                start=(st == 0), stop=(st == n_st - 1),
                                skip_group_check=True,
                            )
                        rc1 = accpool.tile([1, QC], F32, tag="rc1")
                        nc.vector.reciprocal_approx_fast(rc1[:1, :], dps[:1, :])
                        rbc = accpool.tile([128, QC], F32, tag="rbc")
                        nc.gpsimd.partition_broadcast(rbc[:], rc1[:1, :])
                        nc.vector.tensor_mul(yTc[:, h, :], yps[:], rbc[:])
                        if h == 0 and pending is not None:
                            # previous chunk's projection lands here so its
                            # yTc-normalize latency hides under this chunk's
                            # independent attention matmuls
                            emit_proj(*pending)
                            pending = None

                    pending = (qc, yTc)
                emit_proj(*pending)

    nc.compile()
    return nc


def _get_nc():
    if "nc" not in _CACHE:
        _CACHE["nc"] = _build_nc()
    return _CACHE["nc"]


def _bf16(a):
    return np.ascontiguousarray(np.asarray(a, np.float32)).astype(ml_dtypes.bfloat16)


def _in_maps(x, cos, sin, wq, wk, wv, wproj):
    cos = np.asarray(cos, np.float32)
    sin = np.asarray(sin, np.float32)
    cc = np.tile(np.concatenate([cos, cos], axis=1), (1, NH))      # [T, NH*D]
    ss = np.tile(np.concatenate([sin, -sin], axis=1), (1, NH))     # [T, NH*D]
    # partition-major images: contiguous per-partition DMA lines
    cchost = np.ascontiguousarray(cc.reshape(NT, 128, NH * D).transpose(1, 0, 2))
    sshost = np.ascontiguousarray(ss.reshape(NT, 128, NH * D).transpose(1, 0, 2))
    ident = np.eye(128, dtype=np.float32)
    # msk[p, u] = 1 iff u >= p + 384; diagonal block k uses cols [384-128k, ...)
    p = np.arange(128)[:, None]
    u = np.arange(384 + QC)[None, :]
    mask = (u >= p + 384).astype(np.float32)

    x = np.asarray(x, np.float32)
    wq = np.asarray(wq, np.float32)
    wk = np.asarray(wk, np.float32)
    wv = np.asarray(wv, np.float32)
    wpT = np.asarray(wproj, np.float32).T

    maps = []
    for c in range(8):
        b = c // 2
        hs = (c % 2) * NH
        sl = slice(hs * D, (hs + NH) * D)
        # xh[p, tt, ci, u] = x[b][tt*128+u, ci*128+p]
        xhost = x[b].reshape(NT, 128, CT, 128).transpose(3, 0, 2, 1)
        # wqh[p, ci, o] = wq[sl][o, ci*128+p]
        wqhost = wq[sl].T.reshape(CT, 128, NH * D).transpose(1, 0, 2)
        wkhost = wk[sl].T.reshape(CT, 128, NH * D).transpose(1, 0, 2)
        wvhost = wv[sl].T.reshape(CT, 128, NH * D).transpose(1, 0, 2)
        # wph[p, h, c] = wproj.T[sl][h*128+p, c]
        wphost = wpT[sl].reshape(NH, 128, C).transpose(1, 0, 2)
        maps.append({
            "xh": _bf16(xhost),
            "wqh": _bf16(wqhost),
            "wkh": _bf16(wkhost),
            "wvh": _bf16(wvhost),
            "wph": _bf16(wphost),
            "cch": _bf16(cchost),
            "ssh": _bf16(sshost),
            "ident": _bf16(ident),
            "ones_in": _bf16(np.ones((128, 1), dtype=np.float32)),
            "mask_in": _bf16(mask),
        })
    return maps


def kernel(x, cos, sin, wq, wk, wv, wproj, _trace=False):
    nc = _get_nc()
    maps = _in_maps(x, cos, sin, wq, wk, wv, wproj)
    res = run_bass_kernel_spmd(nc, maps, core_ids=list(range(8)), trace=_trace)
    parts = [np.asarray(r["out"], dtype=np.float32) for r in res.results]
    outv = np.stack([parts[2 * b] + parts[2 * b + 1] for b in range(B)])
    if _trace:
        _CACHE["last_results"] = res
    return outv.astype(np.float32)
